# revision 5
# baseline (speedup 1.0000x reference)
"""BCEWithLogitsLoss(mean) over (8192, 8192) logits, data-parallel over 8
NeuronCores (1024 rows each).

Identity: softplus(x) - x*t  =  softplus((1-2t)*x)   for t in {0,1}
so with y = sign-flipped x (flip where j < targets[i], applied on host by
XORing the fp8 sign bit -- exact), the device computes

    loss = mean softplus(y) = mean ln(1 + exp(y))

ACT runs every table function at 1x (~6.4us per [128,8192] pass), so the
ln work is pushed onto the (much faster) DVE as a pairwise product tree:

  sum_i ln(1+e^{y_i}) over a group of 16 = ln( prod_i (1+e^{y_i}) )

Per-core pipeline, one [128, 8192] row-block tile per step:
  SYNC  dma y tile (1 MiB fp8) -> SBUF
  ACT   u = exp(y)  (fp8 in, bf16 out; fp8 input costs nothing extra)
  DVE   w = (u + 1) * S   (one fused tensor_scalar, 4x perf mode; S
        centers the group products inside the Ln table's accurate
        domain, see _LNS below)
        4-level pairwise product tree on halves (tensor_tensor, 2x)
        -> products of 16 factors, [128, 512] per tile
  ACT   one ln per round over the collected [128, 4096] products with
        accum_out -> per-partition sums.  exp and ln share one ACT table
        set (natural_log_exp_and_others): no table switches at all.

Product buffer is double-buffered across rounds so the ln never stalls
the exp cadence.  With S = e^-1.875 a group product is exp(sum of 16
softplus terms - 30): typically e^-17, and +-10 sigma of data stays
within both the Ln table domain (~e^+-44.6) and bf16 range.
Host reduces [128] f32 partials in float64:
    loss = sum / (B*N) - ln(S).
"""

import numpy as np

_B, _N = 8192, 8192
_NCORES = 8
_ROWS = _B // _NCORES  # 1024
_P = 128
_RB = _ROWS // _P  # 8 tiles per core per round
_DEPTH = 4  # product-tree depth: 2**_DEPTH factors per ln input elem
_PW = _N >> _DEPTH  # ln input cols per tile (512 at depth 4)
# The ACT Ln table is only accurate on ~[4e-20, 2.4e19] (about e^+-44.6,
# clamped outside -- measured on device).  Scale each factor w=(1+e^y)*S
# so group products sit mid-domain: product = exp(sum_16 softplus - 30),
# typically e^-17, within +-10 sigma of any plausible randn data.  The
# -16*ln(S) = +30 nats per group are added back on the host.
_LNS = -1.875  # ln(S); S = exp(-1.875) ~ 0.1534

_MODE = "f8"  # "f8": fp8 HBM + ACT reads fp8; "bf16": bf16 HBM

_cache = {}


def _build_nc(repeat=1, mode=None):
    import concourse.bass as bass
    import concourse.mybir as mybir

    mode = mode or _MODE
    f32 = mybir.dt.float32
    bf16 = mybir.dt.bfloat16
    f8 = mybir.dt.float8e4
    F = mybir.ActivationFunctionType
    A = mybir.AluOpType

    hbm_dt = bf16 if mode == "bf16" else f8

    nc = bass.Bass()
    y_d = nc.dram_tensor("y", [_ROWS, _N], hbm_dt, kind="ExternalInput")
    sp_d = nc.dram_tensor("sp_out", [_P, 1], f32, kind="ExternalOutput")

    from contextlib import ExitStack

    with ExitStack() as ctx:
        yt2 = ctx.enter_context(nc.sbuf_tensor([_P, 2 * _N], hbm_dt))  # 2-buf y
        ut2 = ctx.enter_context(nc.sbuf_tensor([_P, 2 * _N], bf16))  # 2-buf exp
        wt2 = ctx.enter_context(nc.sbuf_tensor([_P, 2 * _N], bf16))  # 2-buf 1+exp
        ptmp = [
            ctx.enter_context(
                nc.sbuf_tensor(f"ptmp{d}", [_P, _N >> (d + 1)], bf16)
            )
            for d in range(_DEPTH - 1)
        ]
        # per-round product collection, double-buffered across rounds
        pbig = ctx.enter_context(nc.sbuf_tensor([_P, 2 * _RB * _PW], bf16))
        lnjunk = ctx.enter_context(nc.sbuf_tensor([_P, _RB * _PW], bf16))
        sp_acc = ctx.enter_context(nc.sbuf_tensor([_P, 1], f32))
        dsem0 = ctx.enter_context(nc.semaphore())  # y loads, even tiles
        dsem1 = ctx.enter_context(nc.semaphore())  # y loads, odd tiles
        asem = ctx.enter_context(nc.semaphore())  # exp completions
        tsem = ctx.enter_context(nc.semaphore())  # w=u+1 completions
        p1sem = ctx.enter_context(nc.semaphore())  # p1 completions
        vsem = ctx.enter_context(nc.semaphore())  # p-tree completions
        lsem = ctx.enter_context(nc.semaphore())  # ln completions
        fsem = ctx.enter_context(nc.semaphore())  # final out dma
        block = ctx.enter_context(nc.Block())
        yt = [yt2[:, :_N], yt2[:, _N:]]
        ut = [ut2[:, :_N], ut2[:, _N:]]
        wt = [wt2[:, :_N], wt2[:, _N:]]
        pb = [pbig[:, : _RB * _PW], pbig[:, _RB * _PW :]]

        _T = repeat * _RB

        @block.sync
        def _(sync):
            for t in range(_T):
                rb = t % _RB
                if t >= 2:
                    sync.wait_ge(asem, t - 1)  # yt[t%2] freed by exp t-2
                sync.dma_start(
                    out=yt[t % 2], in_=y_d[rb * _P : (rb + 1) * _P, :]
                ).then_inc(dsem0 if t % 2 == 0 else dsem1, 16)

        @block.scalar
        def _(scalar):
            def emit_ln(r):
                # products of round r all written once vsem >= _RB*(r+1)
                scalar.wait_ge(vsem, _RB * (r + 1))
                nc.scalar.activation(
                    lnjunk[:], pb[r % 2], F.Ln, accum_out=sp_acc[:, 0:1]
                ).then_inc(lsem, 1)

            for t in range(_T):
                r, rb = divmod(t, _RB)
                scalar.wait_ge(dsem0 if t % 2 == 0 else dsem1, 16 * (t // 2 + 1))
                if t >= 2:
                    scalar.wait_ge(tsem, t - 1)  # ut[t%2] freed by w of t-2
                nc.scalar.activation(ut[t % 2], yt[t % 2], F.Exp).then_inc(asem, 1)
                # ln of round r-1 emitted mid-round r: products long done
                if r >= 1 and rb == 3:
                    emit_ln(r - 1)
            emit_ln(repeat - 1)

        @block.vector
        def _(vector):
            nc.vector.memset(sp_acc[:], 0.0)
            for t in range(_T):
                r, rb = divmod(t, _RB)
                vector.wait_ge(asem, t + 1)
                if t >= 2:
                    vector.wait_ge(p1sem, t - 1)  # wt[t%2] freed by p1 of t-2
                if r >= 2 and rb == 0:
                    # pb[r%2] was read by ln of round r-2 (lsem r-1)
                    vector.wait_ge(lsem, r - 1)
                nc.vector.tensor_scalar(
                    out=wt[t % 2],
                    in0=ut[t % 2],
                    scalar1=1.0,
                    scalar2=float(np.exp(_LNS)),
                    op0=A.add,
                    op1=A.mult,
                ).then_inc(tsem, 1)
                src = wt[t % 2]
                for d in range(_DEPTH):
                    h = _N >> (d + 1)
                    dst = (
                        ptmp[d]
                        if d < _DEPTH - 1
                        else pb[r % 2][:, rb * _PW : (rb + 1) * _PW]
                    )
                    ins = nc.vector.tensor_tensor(
                        out=dst[:, :h] if d < _DEPTH - 1 else dst,
                        in0=src[:, :h],
                        in1=src[:, h : 2 * h],
                        op=A.mult,
                    )
                    if d == 0:
                        ins.then_inc(p1sem, 1)
                    src = dst
                ins.then_inc(vsem, 1)

        @block.sync
        def _(sync):
            sync.wait_ge(lsem, repeat)
            sync.dma_start(out=sp_d[:], in_=sp_acc[:]).then_inc(fsem, 16)
            sync.wait_ge(fsem, 16)

    return nc


def _get_nc():
    if "nc" not in _cache:
        _cache["nc"] = _build_nc()
    return _cache["nc"]


def _prep_in_maps(inputs, targets):
    import ml_dtypes

    x = np.asarray(inputs, dtype=np.float32)
    t = np.asarray(targets)
    assert x.shape == (_B, _N) and t.shape == (_B,)
    mask = np.arange(_N, dtype=np.int64)[None, :] < np.asarray(t, np.int64)[:, None]
    if _MODE == "bf16":
        yb = np.ascontiguousarray(x.astype(ml_dtypes.bfloat16))
        yv = yb.view(np.uint16)
        yv ^= mask.astype(np.uint16) << np.uint16(15)
        y = yv.view(ml_dtypes.bfloat16)
    else:
        yb = np.ascontiguousarray(x.astype(ml_dtypes.float8_e4m3fn))
        yv = yb.view(np.uint8)
        yv ^= mask.astype(np.uint8) << np.uint8(7)
        y = yv.view(ml_dtypes.float8_e4m3fn)
    return [
        {"y": np.ascontiguousarray(y[c * _ROWS : (c + 1) * _ROWS])}
        for c in range(_NCORES)
    ]


def kernel(inputs, targets):
    from concourse.bass_utils import run_bass_kernel_spmd

    nc = _get_nc()
    in_maps = _prep_in_maps(inputs, targets)

    res = run_bass_kernel_spmd(nc, in_maps, list(range(_NCORES)))

    total = np.float64(0.0)
    for c in range(_NCORES):
        total += np.sum(res.results[c]["sp_out"].astype(np.float64))
    loss = total / (np.float64(_B) * np.float64(_N)) - np.float64(_LNS)
    return np.float32(loss)
